# revision 15
# baseline (speedup 1.0000x reference)
"""Based-style linear attention (Taylor feature map) on 8 Trainium2 cores.

Math: reference computes, per head h (FDIM=16, HEAD_DIM=64):
    q,k = HS@Wq, HS@Wk    (per-head 16 dims), v = HS@Wv (per-head 64 dims)
    phi(x) = [1, x/2, outer(x,x)/(sqrt(2)*4)]      (273 dims)
    y_t = sum_{s<=t} (phi(q_t).phi(k_s)) v_s / sum_{s<=t} phi(q_t).phi(k_s)
    out = concat_h(y) @ Wo

Key identity: phi(q).phi(k) = 1 + S/4 + S^2/32 where S = q.k (16-dim dot)
            = Square(S/sqrt(32) + 1/sqrt(2)) + 1/2.
So scores come from 16-dim dot products + one ACT Square pass; the 273-dim
feature map is never materialized.

Sharding: head-parallel, no collectives. 16 virtual heads (12 real + 4
zero dummies), 2 per core. Host sums the 8 partial outputs.

v3 structure (all per core, 2 heads):
 - inputs: one packed [128, NWIN] weights/consts tensor (2 DMAs) + hsT in
   12 half-L tiles (lo = t<512 first), issued in parallel from the two
   HWDGE queues (sync + scalar).
 - 12 dummy matmuls on zeros open the PE queue: they run during the input
   DMA wait and warm the PE HAM clock gate (cold 1.2GHz -> warm 2.4GHz).
 - q/k projections use ONE merged 128-col stationary (k_h0|k_h1|q_h0|q_h1
   16-col groups at 32-col offsets) so hs is streamed once, not twice.
 - attention runs in two passes over query columns (t<512, then t>=512)
   so pass A starts as soon as the lo half of hs has landed.
 - score matmuls have K=32: the two heads' stationaries sit at partition
   offsets 0/32 so they row-tile into independent 32x128 PE sub-arrays
   and run concurrently; both heads share one psum bank pair and one ACT
   Square call per (pass, j).
 - nums[h] [65, L] psum: v-cols 0-63 + den ride-along col 64 (ones), +1/2
   causal terms folded in via htri / colsum-sel constant matmuls.
 - divide, pipelined by L-half: reciprocal_approx_fast on nums, K=1
   matmul broadcasts the den-reciprocal row into a [128, L] psum, ACT
   copies -> rb, DVE muls produce yT [128, L] (h0 rows 0-63, h1 64-127).
 - o-proj: yT stacked layout makes it ONE K=128 matmul group per chunk
   (wo is [128, 768] with both heads' rows). psum->sbuf copies alternate
   ACT/DVE; output chunks pair into 4 DMAs on the idle sync queue.
"""

import math

import numpy as np
import ml_dtypes

import concourse.bass as bass
import concourse.mybir as mybir
import concourse.tile as tile
from concourse import bacc
from concourse.bass_utils import run_bass_kernel_spmd

L = 1024
D = 768
H = 12
FD = 16
HD = 64
NCORE = 8
NCH = 8  # L chunks of 128
KB = 6  # contraction blocks of 128 over D
F32 = mybir.dt.float32
BF16 = mybir.dt.bfloat16
DT = BF16

A_SCALE = 1.0 / math.sqrt(32.0)
A_BIAS = 1.0 / math.sqrt(2.0)

# win column map ([128, NWIN] bf16)
WQK0 = 0                 # 6 kb-blocks x 128 (merged qk stationary)
WV0 = WQK0 + KB * 128    # 6 kb-blocks x 130
WINA = WV0 + KB * 130    # end of win_a
WO0 = 0                  # win_b: wo [128, 768] (h0 rows 0-63, h1 64-127)
TRI2_0 = WO0 + D         # [tri | tri] 256
HTRI0 = TRI2_0 + 256     # htri 128
ONES8_0 = HTRI0 + 128    # ones8 64
NWINB = ONES8_0 + 64
NWIN = WINA + NWINB

_compiled_nc = None
_last_in_maps = None


def _build_nc():
    nc = bacc.Bacc("TRN2", target_bir_lowering=False, debug=False, num_devices=NCORE)

    hsT = nc.dram_tensor("hsT", [D, L], DT, kind="ExternalInput")
    win = nc.dram_tensor("win", [128, NWIN], DT, kind="ExternalInput")
    selw = nc.dram_tensor("selw", [8, L], DT, kind="ExternalInput")
    outp = nc.dram_tensor("outp", [L, D], DT, kind="ExternalOutput")

    with tile.TileContext(nc) as tc:
        with (
            tc.tile_pool(name="cst", bufs=1) as cst,
            tc.tile_pool(name="sqp", bufs=5) as sqp,
            tc.tile_pool(name="wrk", bufs=2) as wrk,
        ):
            # ---- PE warm-up: first in the PE queue, runs during DMA wait ----
            warm_sb = cst.tile([128, 512], DT, tag="warm")
            nc.vector.memset(warm_sb, 0.0)
            warm_out = cst.tile([128, 1], F32, tag="warmout")
            with tc.tile_pool(name="psw", bufs=1, space="PSUM") as psw:
                pw = psw.tile([128, 512], F32, tag="pw")
                for i in range(12):
                    nc.tensor.matmul(
                        pw, warm_sb[:, 0:128], warm_sb, start=(i == 0), stop=(i == 11)
                    )
                nc.vector.tensor_copy(warm_out, pw[:, 0:1])

            # ---- input DMAs: split across the two HWDGE queues ----
            hs_re = hsT.ap().rearrange("(po pi) f -> pi po f", pi=128)
            wina_sb = cst.tile([128, WINA], DT, tag="wina")
            winb_sb = cst.tile([128, NWINB], DT, tag="winb")
            sel_sb = cst.tile([8, L], DT, tag="sel")
            hs_lo = [
                cst.tile([128, 512], DT, tag=f"hslo{kb}", name=f"hslo{kb}")
                for kb in range(KB)
            ]
            hs_hi = [
                cst.tile([128, 512], DT, tag=f"hshi{kb}", name=f"hshi{kb}")
                for kb in range(KB)
            ]
            # sync queue
            nc.sync.dma_start(out=wina_sb, in_=win.ap()[:, 0:WINA])
            for kb in (0, 2, 4):
                nc.sync.dma_start(out=hs_lo[kb], in_=hs_re[:, kb, 0:512])
            for kb in (0, 2, 4):
                nc.sync.dma_start(out=hs_hi[kb], in_=hs_re[:, kb, 512:1024])
            # scalar queue
            for kb in (1, 3, 5):
                nc.scalar.dma_start(out=hs_lo[kb], in_=hs_re[:, kb, 0:512])
            nc.scalar.dma_start(out=winb_sb, in_=win.ap()[:, WINA:NWIN])
            for kb in (1, 3, 5):
                nc.scalar.dma_start(out=hs_hi[kb], in_=hs_re[:, kb, 512:1024])
            nc.scalar.dma_start(out=sel_sb, in_=selw.ap())

            def hs(kb, c0, c1):
                if c1 <= 512:
                    return hs_lo[kb][:, c0:c1]
                return hs_hi[kb][:, c0 - 512 : c1 - 512]

            def wqk(kb):
                return wina_sb[:, WQK0 + kb * 128 : WQK0 + (kb + 1) * 128]

            def wv(kb):
                return wina_sb[:, WV0 + kb * 130 : WV0 + (kb + 1) * 130]

            wo_sb = winb_sb[:, WO0 : WO0 + D]
            tri2_sb = winb_sb[:, TRI2_0 : TRI2_0 + 256]
            htri_sb = winb_sb[:, HTRI0 : HTRI0 + 128]
            ones8_sb = winb_sb[:, ONES8_0 : ONES8_0 + 64]

            bias_sb = cst.tile([128, 1], F32, tag="bias")
            nc.vector.memset(bias_sb, A_BIAS)
            # row of ones at partition 64, for the den-reciprocal broadcast
            ones64_sb = cst.tile([65, 64], F32, tag="ones64")
            nc.vector.memset(ones64_sb, 0.0)
            nc.vector.memset(ones64_sb[64:65, :], 1.0)

            kq_sb = cst.tile([64, 2048], DT, tag="kq")
            vx_sb = cst.tile([128, NCH, 130], DT, tag="vx")
            colsum_sb = cst.tile([8, 130], DT, tag="colsum")

            # ================= projections =================
            with tc.tile_pool(name="ps1", bufs=4, space="PSUM") as ps1:
                # q/k -> kq_sb [64, 2048]; partitions 0-15 head0, 32-47 head1
                # (rest zero); cols 0-1023 = k^T, 1024-2047 = q^T
                for half in range(2):
                    ph = ps1.tile([128, 512], F32, tag="pB", name=f"pqk{half}")
                    for kb in range(KB):
                        nc.tensor.matmul(
                            ph,
                            wqk(kb),
                            hs(kb, half * 512, (half + 1) * 512),
                            start=(kb == 0),
                            stop=(kb == KB - 1),
                        )
                    co = half * 512
                    nc.scalar.activation(
                        out=kq_sb[:, co : co + 512],
                        in_=ph[0:64, :],
                        func=mybir.ActivationFunctionType.Copy,
                    )
                    nc.vector.tensor_copy(
                        kq_sb[:, 1024 + co : 1024 + co + 512], ph[64:128, :]
                    )
                # v -> vx_sb [128, 8, 130]: cols 0-63 v_h0, 64 ones,
                # 65-128 v_h1, 129 ones
                for ch in range(NCH):
                    pv = ps1.tile([128, 130], F32, tag="pB", name=f"pv{ch}")
                    for kb in range(KB):
                        nc.tensor.matmul(
                            pv,
                            hs(kb, ch * 128, (ch + 1) * 128),
                            wv(kb),
                            start=(kb == 0),
                            stop=(kb == KB - 1),
                        )
                    nc.vector.tensor_copy(vx_sb[:, ch, :], pv)
                nc.vector.memset(vx_sb[:, :, 64], 1.0)
                nc.vector.memset(vx_sb[:, :, 129], 1.0)

                # per-chunk column sums of vx (inter-chunk +1/2 term)
                pcs = ps1.tile([8, 130], F32, tag="pB", name="pcs")
                for ch in range(NCH):
                    nc.tensor.matmul(
                        pcs,
                        ones8_sb[:, ch * 8 : (ch + 1) * 8],
                        vx_sb[:, ch, :],
                        start=(ch == 0),
                        stop=(ch == NCH - 1),
                    )
                nc.vector.tensor_copy(colsum_sb, pcs)

            # ================= attention =================
            yT_sb = cst.tile([128, L], DT, tag="yT")
            with tc.tile_pool(name="psnum", bufs=1, space="PSUM") as psnum:
                nums = [
                    psnum.tile([65, L], F32, tag=f"pN{h}", name=f"num{h}")
                    for h in range(2)
                ]
                rb = wrk.tile([128, L], F32, tag="rb")
                rcs = [
                    wrk.tile([65, L], F32, tag="rc", name=f"rc{h}") for h in range(2)
                ]

                def finalize_piece(psa, p):
                    """cols [256p, 256p+256) of nums are complete: fold in the
                    inter-chunk +1/2 term (colsum-sel) and start the den
                    reciprocal on the DVE, pipelined under the j-loops."""
                    a, b = 256 * p, 256 * p + 256
                    for h in range(2):
                        nc.tensor.matmul(
                            nums[h][:, a:b],
                            colsum_sb[:, 65 * h : 65 * h + 65],
                            sel_sb[:, a:b],
                            start=False,
                            stop=True,
                        )
                    for h in range(2):
                        nc.vector.reciprocal_approx_fast(
                            out=rcs[h][:, a:b], in_=nums[h][:, a:b]
                        )

                def divide_half(pool, half):
                    a, b = 512 * half, 512 * half + 512
                    prb = pool.tile(
                        [128, 512], F32, tag="pA" if half == 0 else "prb",
                        name=f"prb_h{half}",
                    )
                    for h in range(2):
                        nc.tensor.matmul(
                            prb[64 * h : 64 * h + 64, :],
                            ones64_sb[64:65, :],
                            rcs[h][64:65, a:b],
                            start=True,
                            stop=True,
                        )
                    nc.scalar.activation(
                        out=rb[:, a:b],
                        in_=prb,
                        func=mybir.ActivationFunctionType.Copy,
                    )
                    for h in range(2):
                        nc.vector.tensor_mul(
                            yT_sb[64 * h : 64 * h + 64, a:b],
                            nums[h][0:64, a:b],
                            rb[64 * h : 64 * h + 64, a:b],
                        )

                sq_t = {}
                with tc.tile_pool(name="psa", bufs=2, space="PSUM") as psa:
                    # pass A: query cols t in [tlo, 512) for kv-chunks 0-3
                    for j in range(4):
                        tlo = j * 128
                        w = 512 - tlo
                        sq = sqp.tile([128, 2, L], DT, tag="sq", name=f"sq{j}")
                        sq_t[j] = sq
                        pa = psa.tile([128, 2, 512], F32, tag="pA", name=f"paA{j}")
                        for h in range(2):
                            nc.tensor.matmul(
                                pa[:, h, :w],
                                kq_sb[32 * h : 32 * h + 32, tlo : tlo + 128],
                                kq_sb[
                                    32 * h : 32 * h + 32, 1024 + tlo : 1024 + 512
                                ],
                                start=True,
                                stop=True,
                            )
                        nc.scalar.activation(
                            out=sq[:, :, :w],
                            in_=pa[:, :, :w],
                            func=mybir.ActivationFunctionType.Square,
                            scale=A_SCALE,
                            bias=bias_sb,
                        )
                        # diagonal blocks: fold the +1/2 term and the causal
                        # mask into one op: sq' = (sq + 0.5) * tri
                        nc.vector.scalar_tensor_tensor(
                            out=sq[:, :, 0:128],
                            in0=sq[:, :, 0:128],
                            scalar=0.5,
                            in1=tri2_sb,
                            op0=mybir.AluOpType.add,
                            op1=mybir.AluOpType.mult,
                        )
                        for h in range(2):
                            nc.tensor.matmul(
                                nums[h][:, tlo:512],
                                vx_sb[:, j, 65 * h : 65 * h + 65],
                                sq[:, h, 0:w],
                                start=(j == 0),
                                stop=False,
                            )
                        if j == 1:
                            finalize_piece(psa, 0)
                        elif j == 3:
                            finalize_piece(psa, 1)
                    # pass B: query cols t in [512, 1024) (j<4) or full (j>=4)
                    for j in range(NCH):
                        tlo = j * 128
                        if j < 4:
                            sq = sq_t[j]
                            c0, w = 512 - tlo, 512
                            qlo = 1024 + 512
                        else:
                            sq = sqp.tile([128, 2, L], DT, tag="sq", name=f"sq{j}")
                            c0, w = 0, L - tlo
                            qlo = 1024 + tlo
                        pa = psa.tile([128, 2, 512], F32, tag="pA", name=f"paB{j}")
                        for h in range(2):
                            nc.tensor.matmul(
                                pa[:, h, :w],
                                kq_sb[32 * h : 32 * h + 32, tlo : tlo + 128],
                                kq_sb[32 * h : 32 * h + 32, qlo : qlo + w],
                                start=True,
                                stop=True,
                            )
                        nc.scalar.activation(
                            out=sq[:, :, c0 : c0 + w],
                            in_=pa[:, :, :w],
                            func=mybir.ActivationFunctionType.Square,
                            scale=A_SCALE,
                            bias=bias_sb,
                        )
                        if j >= 4:
                            nc.vector.scalar_tensor_tensor(
                                out=sq[:, :, 0:128],
                                in0=sq[:, :, 0:128],
                                scalar=0.5,
                                in1=tri2_sb,
                                op0=mybir.AluOpType.add,
                                op1=mybir.AluOpType.mult,
                            )
                        for h in range(2):
                            nc.tensor.matmul(
                                nums[h][:, max(tlo, 512) : 1024],
                                vx_sb[:, j, 65 * h : 65 * h + 65],
                                sq[:, h, c0 : c0 + w],
                                start=(j == 0),
                                stop=False,
                            )
                        if j == 0:
                            # pieces 0-1 (t<512) are final: divide half 0
                            # here, hidden under the pass-B work
                            divide_half(psa, 0)
                        elif j == 5:
                            finalize_piece(psa, 2)
                        elif j == 7:
                            finalize_piece(psa, 3)

                # half 1 of the divide; its reciprocals (pieces 2-3) were
                # started under the j-loop
                with tc.tile_pool(name="ps2", bufs=1, space="PSUM") as ps2:
                    divide_half(ps2, 1)

            # ================= output projection =================
            out_re = outp.ap().rearrange("(c p) d -> p c d", p=128)
            with tc.tile_pool(name="ps3", bufs=4, space="PSUM") as ps3:
                for p in range(NCH // 2):
                    osb = wrk.tile([128, 2, D], DT, tag="osb", name=f"osb{p}")
                    for s in range(2):
                        i = 2 * p + s
                        po = ps3.tile([128, D], F32, tag="po", name=f"po{i}")
                        for a, b in ((0, 512), (512, 768)):
                            nc.tensor.matmul(
                                po[:, a:b],
                                yT_sb[:, i * 128 : (i + 1) * 128],
                                wo_sb[:, a:b],
                                start=True,
                                stop=True,
                            )
                        if i % 2 == 1:
                            nc.scalar.activation(
                                out=osb[:, s, :],
                                in_=po,
                                func=mybir.ActivationFunctionType.Copy,
                            )
                        else:
                            nc.vector.tensor_copy(osb[:, s, :], po)
                    if p % 2 == 0:
                        nc.sync.dma_start(out=out_re[:, 2 * p : 2 * p + 2, :], in_=osb)
                    else:
                        nc.scalar.dma_start(
                            out=out_re[:, 2 * p : 2 * p + 2, :], in_=osb
                        )

    nc.finalize()
    return nc


def _host_consts():
    s = np.arange(128)[:, None]
    t = np.arange(128)[None, :]
    tri = (s <= t).astype(np.float32)
    htri = 0.5 * tri
    sel = np.zeros((8, L), dtype=np.float32)
    for i in range(8):
        sel[:i, i * 128 : (i + 1) * 128] = 0.5
    ones8 = np.zeros((128, 64), dtype=np.float32)
    for ch in range(8):
        ones8[:, ch * 8 + ch] = 1.0
    return tri, htri, sel, ones8


def kernel(hidden_states, Wq, Wk, Wv, Wo):
    global _compiled_nc, _last_in_maps
    hs = np.asarray(hidden_states, dtype=np.float32)[0]  # [L, D]
    Wq = np.asarray(Wq, dtype=np.float32)
    Wk = np.asarray(Wk, dtype=np.float32)
    Wv = np.asarray(Wv, dtype=np.float32)
    Wo = np.asarray(Wo, dtype=np.float32)

    if _compiled_nc is None:
        _compiled_nc = _build_nc()
    nc = _compiled_nc

    bf = ml_dtypes.bfloat16
    hsT = np.ascontiguousarray(hs.T).astype(bf)  # [D, L]
    tri, htri, sel, ones8 = _host_consts()

    in_maps = []
    for c in range(NCORE):
        heads = [2 * c, 2 * c + 1]
        wqk_c = np.zeros((D, 128), dtype=np.float32)
        wv_c = np.zeros((D, 130), dtype=np.float32)
        wo_c = np.zeros((128, D), dtype=np.float32)
        for hi, h in enumerate(heads):
            if h >= H:
                continue
            wqk_c[:, 32 * hi : 32 * hi + FD] = Wk[:, h * FD : (h + 1) * FD]
            wqk_c[:, 64 + 32 * hi : 64 + 32 * hi + FD] = Wq[:, h * FD : (h + 1) * FD]
            wv_c[:, 65 * hi : 65 * hi + HD] = Wv[:, h * HD : (h + 1) * HD]
            wo_c[64 * hi : 64 * hi + HD, :] = Wo[h * HD : (h + 1) * HD, :]
        win_c = np.zeros((128, NWIN), dtype=np.float32)
        # wqk: [768, 128] -> [6, 128p, 128c] -> win[p, kb*128+c]
        win_c[:, WQK0 : WQK0 + KB * 128] = (
            wqk_c.reshape(KB, 128, 128).transpose(1, 0, 2).reshape(128, KB * 128)
        )
        win_c[:, WV0 : WV0 + KB * 130] = (
            wv_c.reshape(KB, 128, 130).transpose(1, 0, 2).reshape(128, KB * 130)
        )
        wb = WINA
        win_c[:, wb + WO0 : wb + WO0 + D] = wo_c
        win_c[:, wb + TRI2_0 : wb + TRI2_0 + 128] = tri
        win_c[:, wb + TRI2_0 + 128 : wb + TRI2_0 + 256] = tri
        win_c[:, wb + HTRI0 : wb + HTRI0 + 128] = htri
        win_c[:, wb + ONES8_0 : wb + ONES8_0 + 64] = ones8
        in_maps.append(
            {
                "hsT": hsT,
                "win": win_c.astype(bf),
                "selw": sel.astype(bf),
            }
        )

    _last_in_maps = in_maps
    res = run_bass_kernel_spmd(nc, in_maps, list(range(NCORE)))
    acc = np.zeros((L, D), dtype=np.float32)
    for c in range(NCORE):
        acc += np.asarray(res.results[c]["outp"], dtype=np.float32)
    return acc.reshape(1, L, D)


# revision 16
# speedup vs baseline: 1.1153x; 1.1153x over previous
"""Based-style linear attention (Taylor feature map) on 8 Trainium2 cores.

Math: reference computes, per head h (FDIM=16, HEAD_DIM=64):
    q,k = HS@Wq, HS@Wk    (per-head 16 dims), v = HS@Wv (per-head 64 dims)
    phi(x) = [1, x/2, outer(x,x)/(sqrt(2)*4)]      (273 dims)
    y_t = sum_{s<=t} (phi(q_t).phi(k_s)) v_s / sum_{s<=t} phi(q_t).phi(k_s)
    out = concat_h(y) @ Wo

Key identity: phi(q).phi(k) = 1 + S/4 + S^2/32 where S = q.k (16-dim dot)
            = Square(S/sqrt(32) + 1/sqrt(2)) + 1/2.
So scores come from 16-dim dot products + one ACT Square pass; the 273-dim
feature map is never materialized.

Sharding: head-parallel, no collectives. 16 virtual heads (12 real + 4
zero dummies), 2 per core. Host sums the 8 partial outputs.

v3 structure (all per core, 2 heads):
 - inputs: one packed [128, NWIN] weights/consts tensor (2 DMAs) + hsT in
   12 half-L tiles (lo = t<512 first), issued in parallel from the two
   HWDGE queues (sync + scalar).
 - 12 dummy matmuls on zeros open the PE queue: they run during the input
   DMA wait and warm the PE HAM clock gate (cold 1.2GHz -> warm 2.4GHz).
 - q/k projections use ONE merged 128-col stationary (k_h0|k_h1|q_h0|q_h1
   16-col groups at 32-col offsets) so hs is streamed once, not twice.
 - attention runs in two passes over query columns (t<512, then t>=512)
   so pass A starts as soon as the lo half of hs has landed.
 - score matmuls have K=32: the two heads' stationaries sit at partition
   offsets 0/32 so they row-tile into independent 32x128 PE sub-arrays
   and run concurrently; both heads share one psum bank pair and one ACT
   Square call per (pass, j).
 - nums[h] [65, L] psum: v-cols 0-63 + den ride-along col 64 (ones), +1/2
   causal terms folded in via htri / colsum-sel constant matmuls.
 - divide, pipelined by L-half: reciprocal_approx_fast on nums, K=1
   matmul broadcasts the den-reciprocal row into a [128, L] psum, ACT
   copies -> rb, DVE muls produce yT [128, L] (h0 rows 0-63, h1 64-127).
 - o-proj: yT stacked layout makes it ONE K=128 matmul group per chunk
   (wo is [128, 768] with both heads' rows). psum->sbuf copies alternate
   ACT/DVE; output chunks pair into 4 DMAs on the idle sync queue.
"""

import math

import numpy as np
import ml_dtypes

import concourse.bass as bass
import concourse.mybir as mybir
import concourse.tile as tile
from concourse import bacc
from concourse.bass_utils import run_bass_kernel_spmd

L = 1024
D = 768
H = 12
FD = 16
HD = 64
NCORE = 8
NCH = 8  # L chunks of 128
KB = 6  # contraction blocks of 128 over D
F32 = mybir.dt.float32
BF16 = mybir.dt.bfloat16
DT = BF16

A_SCALE = 1.0 / math.sqrt(32.0)
A_BIAS = 1.0 / math.sqrt(2.0)

# win column map ([128, NWIN] bf16)
WQK0 = 0                 # 6 kb-blocks x 128 (merged qk stationary)
WV0 = WQK0 + KB * 128    # 6 kb-blocks x 130
WINA = WV0 + KB * 130    # end of win_a
WO0 = 0                  # win_b: wo [128, 768] (h0 rows 0-63, h1 64-127)
TRI2_0 = WO0 + D         # [tri | tri] 256
HTRI0 = TRI2_0 + 256     # htri 128
ONES8_0 = HTRI0 + 128    # ones8 64
NWINB = ONES8_0 + 64
NWIN = WINA + NWINB

_compiled_nc = None
_last_in_maps = None


def _build_nc():
    nc = bacc.Bacc("TRN2", target_bir_lowering=False, debug=False, num_devices=NCORE)

    hsT = nc.dram_tensor("hsT", [D, L], DT, kind="ExternalInput")
    win = nc.dram_tensor("win", [128, NWIN], DT, kind="ExternalInput")
    selw = nc.dram_tensor("selw", [8, L], DT, kind="ExternalInput")
    outp = nc.dram_tensor("outp", [L, D], DT, kind="ExternalOutput")

    with tile.TileContext(nc) as tc:
        with (
            tc.tile_pool(name="cst", bufs=1) as cst,
            tc.tile_pool(name="sqp", bufs=5) as sqp,
            tc.tile_pool(name="wrk", bufs=2) as wrk,
        ):
            # ---- PE warm-up: first in the PE queue, runs during DMA wait ----
            warm_sb = cst.tile([128, 512], DT, tag="warm")
            nc.vector.memset(warm_sb, 0.0)
            warm_out = cst.tile([128, 1], F32, tag="warmout")
            with tc.tile_pool(name="psw", bufs=1, space="PSUM") as psw:
                pw = psw.tile([128, 512], F32, tag="pw")
                for i in range(12):
                    nc.tensor.matmul(
                        pw, warm_sb[:, 0:128], warm_sb, start=(i == 0), stop=(i == 11)
                    )
                nc.vector.tensor_copy(warm_out, pw[:, 0:1])

            # ---- input DMAs: split across the two HWDGE queues ----
            hs_re = hsT.ap().rearrange("(po pi) f -> pi po f", pi=128)
            wina_sb = cst.tile([128, WINA], DT, tag="wina")
            winb_sb = cst.tile([128, NWINB], DT, tag="winb")
            sel_sb = cst.tile([8, L], DT, tag="sel")
            hs_lo = [
                cst.tile([128, 512], DT, tag=f"hslo{kb}", name=f"hslo{kb}")
                for kb in range(KB)
            ]
            hs_hi = [
                cst.tile([128, 512], DT, tag=f"hshi{kb}", name=f"hshi{kb}")
                for kb in range(KB)
            ]
            # sync queue
            nc.sync.dma_start(out=wina_sb, in_=win.ap()[:, 0:WINA])
            for kb in (0, 2, 4):
                nc.sync.dma_start(out=hs_lo[kb], in_=hs_re[:, kb, 0:512])
            for kb in (0, 2, 4):
                nc.sync.dma_start(out=hs_hi[kb], in_=hs_re[:, kb, 512:1024])
            # scalar queue
            for kb in (1, 3, 5):
                nc.scalar.dma_start(out=hs_lo[kb], in_=hs_re[:, kb, 0:512])
            nc.scalar.dma_start(out=winb_sb, in_=win.ap()[:, WINA:NWIN])
            for kb in (1, 3, 5):
                nc.scalar.dma_start(out=hs_hi[kb], in_=hs_re[:, kb, 512:1024])
            nc.scalar.dma_start(out=sel_sb, in_=selw.ap())

            def hs(kb, c0, c1):
                if c1 <= 512:
                    return hs_lo[kb][:, c0:c1]
                return hs_hi[kb][:, c0 - 512 : c1 - 512]

            def wqk(kb):
                return wina_sb[:, WQK0 + kb * 128 : WQK0 + (kb + 1) * 128]

            def wv(kb):
                return wina_sb[:, WV0 + kb * 130 : WV0 + (kb + 1) * 130]

            wo_sb = winb_sb[:, WO0 : WO0 + D]
            tri2_sb = winb_sb[:, TRI2_0 : TRI2_0 + 256]
            htri_sb = winb_sb[:, HTRI0 : HTRI0 + 128]
            ones8_sb = winb_sb[:, ONES8_0 : ONES8_0 + 64]

            bias_sb = cst.tile([128, 1], F32, tag="bias")
            nc.vector.memset(bias_sb, A_BIAS)
            # row of ones at partition 64, for the den-reciprocal broadcast
            ones64_sb = cst.tile([65, 64], F32, tag="ones64")
            nc.vector.memset(ones64_sb, 0.0)
            nc.vector.memset(ones64_sb[64:65, :], 1.0)

            kq_sb = cst.tile([64, 2048], DT, tag="kq")
            vx_sb = cst.tile([128, NCH, 130], DT, tag="vx")
            colsum_sb = cst.tile([8, 130], DT, tag="colsum")

            # ================= projections =================
            with tc.tile_pool(name="ps1", bufs=4, space="PSUM") as ps1:
                # q/k -> kq_sb [64, 2048]; partitions 0-15 head0, 32-47 head1
                # (rest zero); cols 0-1023 = k^T, 1024-2047 = q^T
                for half in range(2):
                    ph = ps1.tile([128, 512], F32, tag="pB", name=f"pqk{half}")
                    for kb in range(KB):
                        nc.tensor.matmul(
                            ph,
                            wqk(kb),
                            hs(kb, half * 512, (half + 1) * 512),
                            start=(kb == 0),
                            stop=(kb == KB - 1),
                        )
                    co = half * 512
                    nc.scalar.activation(
                        out=kq_sb[:, co : co + 512],
                        in_=ph[0:64, :],
                        func=mybir.ActivationFunctionType.Copy,
                    )
                    nc.vector.tensor_copy(
                        kq_sb[:, 1024 + co : 1024 + co + 512], ph[64:128, :]
                    )
                # v -> vx_sb [128, 8, 130]: cols 0-63 v_h0, 64 ones,
                # 65-128 v_h1, 129 ones
                for ch in range(NCH):
                    pv = ps1.tile([128, 130], F32, tag="pB", name=f"pv{ch}")
                    for kb in range(KB):
                        nc.tensor.matmul(
                            pv,
                            hs(kb, ch * 128, (ch + 1) * 128),
                            wv(kb),
                            start=(kb == 0),
                            stop=(kb == KB - 1),
                        )
                    nc.vector.tensor_copy(vx_sb[:, ch, :], pv)
                nc.vector.memset(vx_sb[:, :, 64], 1.0)
                nc.vector.memset(vx_sb[:, :, 129], 1.0)

                # per-chunk column sums of vx (inter-chunk +1/2 term)
                pcs = ps1.tile([8, 130], F32, tag="pB", name="pcs")
                for ch in range(NCH):
                    nc.tensor.matmul(
                        pcs,
                        ones8_sb[:, ch * 8 : (ch + 1) * 8],
                        vx_sb[:, ch, :],
                        start=(ch == 0),
                        stop=(ch == NCH - 1),
                    )
                nc.vector.tensor_copy(colsum_sb, pcs)

            # ================= attention =================
            yT_sb = cst.tile([128, L], DT, tag="yT")
            with tc.tile_pool(name="psnum", bufs=1, space="PSUM") as psnum:
                nums = [
                    psnum.tile([65, L], F32, tag=f"pN{h}", name=f"num{h}")
                    for h in range(2)
                ]
                rb = wrk.tile([128, L], F32, tag="rb")
                rcs = [
                    wrk.tile([65, L], F32, tag="rc", name=f"rc{h}") for h in range(2)
                ]

                def finalize_piece(psa, p):
                    """cols [256p, 256p+256) of nums are complete: fold in the
                    inter-chunk +1/2 term (colsum-sel) and start the den
                    reciprocal on the DVE, pipelined under the j-loops."""
                    a, b = 256 * p, 256 * p + 256
                    for h in range(2):
                        nc.tensor.matmul(
                            nums[h][:, a:b],
                            colsum_sb[:, 65 * h : 65 * h + 65],
                            sel_sb[:, a:b],
                            start=False,
                            stop=True,
                        )
                    for h in range(2):
                        nc.vector.reciprocal_approx_fast(
                            out=rcs[h][:, a:b], in_=nums[h][:, a:b]
                        )

                def divide_half(pool, half):
                    a, b = 512 * half, 512 * half + 512
                    prb = pool.tile([128, 512], F32, tag="prb", name=f"prb_h{half}")
                    for h in range(2):
                        nc.tensor.matmul(
                            prb[64 * h : 64 * h + 64, :],
                            ones64_sb[64:65, :],
                            rcs[h][64:65, a:b],
                            start=True,
                            stop=True,
                        )
                    nc.scalar.activation(
                        out=rb[:, a:b],
                        in_=prb,
                        func=mybir.ActivationFunctionType.Copy,
                    )
                    for h in range(2):
                        nc.vector.tensor_mul(
                            yT_sb[64 * h : 64 * h + 64, a:b],
                            nums[h][0:64, a:b],
                            rb[64 * h : 64 * h + 64, a:b],
                        )

                sq_t = {}
                with tc.tile_pool(name="psa", bufs=2, space="PSUM") as psa:
                    # pass A: query cols t in [tlo, 512) for kv-chunks 0-3
                    for j in range(4):
                        tlo = j * 128
                        w = 512 - tlo
                        sq = sqp.tile([128, 2, L], DT, tag="sq", name=f"sq{j}")
                        sq_t[j] = sq
                        pa = psa.tile([128, 2, 512], F32, tag="pA", name=f"paA{j}")
                        for h in range(2):
                            nc.tensor.matmul(
                                pa[:, h, :w],
                                kq_sb[32 * h : 32 * h + 32, tlo : tlo + 128],
                                kq_sb[
                                    32 * h : 32 * h + 32, 1024 + tlo : 1024 + 512
                                ],
                                start=True,
                                stop=True,
                            )
                        nc.scalar.activation(
                            out=sq[:, :, :w],
                            in_=pa[:, :, :w],
                            func=mybir.ActivationFunctionType.Square,
                            scale=A_SCALE,
                            bias=bias_sb,
                        )
                        # diagonal blocks: fold the +1/2 term and the causal
                        # mask into one op: sq' = (sq + 0.5) * tri
                        nc.vector.scalar_tensor_tensor(
                            out=sq[:, :, 0:128],
                            in0=sq[:, :, 0:128],
                            scalar=0.5,
                            in1=tri2_sb,
                            op0=mybir.AluOpType.add,
                            op1=mybir.AluOpType.mult,
                        )
                        for h in range(2):
                            nc.tensor.matmul(
                                nums[h][:, tlo:512],
                                vx_sb[:, j, 65 * h : 65 * h + 65],
                                sq[:, h, 0:w],
                                start=(j == 0),
                                stop=False,
                            )
                        if j == 1:
                            finalize_piece(psa, 0)
                        elif j == 3:
                            finalize_piece(psa, 1)
                    # pass B: query cols t in [512, 1024) (j<4) or full (j>=4)
                    for j in range(NCH):
                        tlo = j * 128
                        if j < 4:
                            sq = sq_t[j]
                            c0, w = 512 - tlo, 512
                            qlo = 1024 + 512
                        else:
                            sq = sqp.tile([128, 2, L], DT, tag="sq", name=f"sq{j}")
                            c0, w = 0, L - tlo
                            qlo = 1024 + tlo
                        pa = psa.tile([128, 2, 512], F32, tag="pA", name=f"paB{j}")
                        for h in range(2):
                            nc.tensor.matmul(
                                pa[:, h, :w],
                                kq_sb[32 * h : 32 * h + 32, tlo : tlo + 128],
                                kq_sb[32 * h : 32 * h + 32, qlo : qlo + w],
                                start=True,
                                stop=True,
                            )
                        nc.scalar.activation(
                            out=sq[:, :, c0 : c0 + w],
                            in_=pa[:, :, :w],
                            func=mybir.ActivationFunctionType.Square,
                            scale=A_SCALE,
                            bias=bias_sb,
                        )
                        if j >= 4:
                            nc.vector.scalar_tensor_tensor(
                                out=sq[:, :, 0:128],
                                in0=sq[:, :, 0:128],
                                scalar=0.5,
                                in1=tri2_sb,
                                op0=mybir.AluOpType.add,
                                op1=mybir.AluOpType.mult,
                            )
                        for h in range(2):
                            nc.tensor.matmul(
                                nums[h][:, max(tlo, 512) : 1024],
                                vx_sb[:, j, 65 * h : 65 * h + 65],
                                sq[:, h, c0 : c0 + w],
                                start=(j == 0),
                                stop=False,
                            )
                        if j == 5:
                            finalize_piece(psa, 2)
                        elif j == 7:
                            finalize_piece(psa, 3)

                # divide by den; the reciprocals were computed
                # piece-by-piece under the attention loops
                with tc.tile_pool(name="ps2", bufs=2, space="PSUM") as ps2:
                    divide_half(ps2, 0)
                    divide_half(ps2, 1)

            # ================= output projection =================
            out_re = outp.ap().rearrange("(c p) d -> p c d", p=128)
            with tc.tile_pool(name="ps3", bufs=4, space="PSUM") as ps3:
                for p in range(NCH // 2):
                    osb = wrk.tile([128, 2, D], DT, tag="osb", name=f"osb{p}")
                    for s in range(2):
                        i = 2 * p + s
                        po = ps3.tile([128, D], F32, tag="po", name=f"po{i}")
                        for a, b in ((0, 512), (512, 768)):
                            nc.tensor.matmul(
                                po[:, a:b],
                                yT_sb[:, i * 128 : (i + 1) * 128],
                                wo_sb[:, a:b],
                                start=True,
                                stop=True,
                            )
                        if i % 2 == 1:
                            nc.scalar.activation(
                                out=osb[:, s, :],
                                in_=po,
                                func=mybir.ActivationFunctionType.Copy,
                            )
                        else:
                            nc.vector.tensor_copy(osb[:, s, :], po)
                    if p % 2 == 0:
                        nc.sync.dma_start(out=out_re[:, 2 * p : 2 * p + 2, :], in_=osb)
                    else:
                        nc.scalar.dma_start(
                            out=out_re[:, 2 * p : 2 * p + 2, :], in_=osb
                        )

    nc.finalize()
    return nc


def _host_consts():
    s = np.arange(128)[:, None]
    t = np.arange(128)[None, :]
    tri = (s <= t).astype(np.float32)
    htri = 0.5 * tri
    sel = np.zeros((8, L), dtype=np.float32)
    for i in range(8):
        sel[:i, i * 128 : (i + 1) * 128] = 0.5
    ones8 = np.zeros((128, 64), dtype=np.float32)
    for ch in range(8):
        ones8[:, ch * 8 + ch] = 1.0
    return tri, htri, sel, ones8


def kernel(hidden_states, Wq, Wk, Wv, Wo):
    global _compiled_nc, _last_in_maps
    hs = np.asarray(hidden_states, dtype=np.float32)[0]  # [L, D]
    Wq = np.asarray(Wq, dtype=np.float32)
    Wk = np.asarray(Wk, dtype=np.float32)
    Wv = np.asarray(Wv, dtype=np.float32)
    Wo = np.asarray(Wo, dtype=np.float32)

    if _compiled_nc is None:
        _compiled_nc = _build_nc()
    nc = _compiled_nc

    bf = ml_dtypes.bfloat16
    hsT = np.ascontiguousarray(hs.T).astype(bf)  # [D, L]
    tri, htri, sel, ones8 = _host_consts()

    in_maps = []
    for c in range(NCORE):
        heads = [2 * c, 2 * c + 1]
        wqk_c = np.zeros((D, 128), dtype=np.float32)
        wv_c = np.zeros((D, 130), dtype=np.float32)
        wo_c = np.zeros((128, D), dtype=np.float32)
        for hi, h in enumerate(heads):
            if h >= H:
                continue
            wqk_c[:, 32 * hi : 32 * hi + FD] = Wk[:, h * FD : (h + 1) * FD]
            wqk_c[:, 64 + 32 * hi : 64 + 32 * hi + FD] = Wq[:, h * FD : (h + 1) * FD]
            wv_c[:, 65 * hi : 65 * hi + HD] = Wv[:, h * HD : (h + 1) * HD]
            wo_c[64 * hi : 64 * hi + HD, :] = Wo[h * HD : (h + 1) * HD, :]
        win_c = np.zeros((128, NWIN), dtype=np.float32)
        # wqk: [768, 128] -> [6, 128p, 128c] -> win[p, kb*128+c]
        win_c[:, WQK0 : WQK0 + KB * 128] = (
            wqk_c.reshape(KB, 128, 128).transpose(1, 0, 2).reshape(128, KB * 128)
        )
        win_c[:, WV0 : WV0 + KB * 130] = (
            wv_c.reshape(KB, 128, 130).transpose(1, 0, 2).reshape(128, KB * 130)
        )
        wb = WINA
        win_c[:, wb + WO0 : wb + WO0 + D] = wo_c
        win_c[:, wb + TRI2_0 : wb + TRI2_0 + 128] = tri
        win_c[:, wb + TRI2_0 + 128 : wb + TRI2_0 + 256] = tri
        win_c[:, wb + HTRI0 : wb + HTRI0 + 128] = htri
        win_c[:, wb + ONES8_0 : wb + ONES8_0 + 64] = ones8
        in_maps.append(
            {
                "hsT": hsT,
                "win": win_c.astype(bf),
                "selw": sel.astype(bf),
            }
        )

    _last_in_maps = in_maps
    res = run_bass_kernel_spmd(nc, in_maps, list(range(NCORE)))
    acc = np.zeros((L, D), dtype=np.float32)
    for c in range(NCORE):
        acc += np.asarray(res.results[c]["outp"], dtype=np.float32)
    return acc.reshape(1, L, D)
